# revision 5
# baseline (speedup 1.0000x reference)
"""ConvLSTM block Trainium2 kernel (8 NeuronCores).

Sharding: 8 cores = 4 batches x 2 H-halves. Bottom-half cores process their
slab vertically flipped (with kh-flipped conv kernels) so one SPMD program
serves all cores.

Halo scheme: the cores exchange a 4-row halo of (h, c) every 4 steps (after
steps 3, 7, 11 -> 3 AllReduces of ~10us, all hidden) instead of one row
every step. Between exchanges each core redundantly computes a shrinking
wedge of the neighbor's rows (3, 2, 1, 0 extra rows per step) in a small
"mini" PSUM group, keeping all owned rows exact at every step.

Conv structure: 7 matmul passes per (gate-tile, chunk) PSUM group instead
of the naive 9, by packing the contraction dim fully:
  P1-3: [h(r-1) ; h(r)] dup-copy pair, col windows -1/0/+1   (K=128)
  P4:   [h(r+1,c=-1) ; h(r+1,c=0)] col-baked copies QA       (K=128)
  P5:   [h(r+1,c=+1) ; x taps(j0,d0),(j0,d1)] QB             (K=128)
  P6:   x taps (j0,d2),(j1,d0),(j1,d1),(j1,d2)               (K=128)
  P7:   x taps (j2,d0),(j2,d1),(j2,d2)                       (K=96)
where the x tensor is pre-extracted on host into the stride-2 compact
domain (xbk[t, 32*(3j+d)+ch, q, w] = x[ch, 2q+j, 2w+d]) so x rows share
the stride-1 free geometry of h and can co-habit passes with it.
The three col-baked copies of h(r+1) are written per chunk on the
Scalar/Vector/GpSimd queues (one each) to balance engine load.

Gates/LSTM/BN run on ACT+DVE out of PSUM; bias and BN are folded into the
activations. On exchange steps the boundary chunk runs first so the
AllReduce is in flight ~5us into the step; its consumers (next step's
boundary chunk + mini) run ~23us later, hiding the collective. The mini
group lives in its own 2-bank PSUM pool so its reuse cannot serialize the
main 6-bank chunk ring.
"""
import os
import numpy as np

T, H2, W2, F, CIN = 16, 64, 64, 64, 32
WP, NQ = 66, 8
N_HALVES = 2
R = H2 // N_HALVES
E = 4                 # halo width / steps per exchange phase
EXT = E - 1           # max redundant rows per step
RX = R + EXT          # rows of conv output prepared per step
SLAB = 2 * RX + 1     # input rows needed per slab
HSLOT = R + 6         # h row-slot count (pad + R owned + ext/halo + pad)
NCHUNK = R // NQ
MM_DT = os.environ.get("CONV_LSTM_MM_DT", "bf16")  # bf16 | fp32 | fp32r

_CACHE = {}


def _storage_np_dtype():
    import ml_dtypes
    return ml_dtypes.bfloat16 if MM_DT == "bf16" else np.float32


def _prep_core_inputs(x, W, U, b, gamma, beta, moving_mean, moving_var,
                      bidx, half):
    sdt = _storage_np_dtype()
    flip = (half == 1)

    # x slab [T, CIN, SLAB, 130]; XLA SAME (stride2,k3,even) pads
    # bottom/right only: out row r reads input rows 2r..2r+2.
    xs = np.zeros((T, CIN, SLAB, 130), np.float32)
    xc = np.ascontiguousarray(x[bidx].transpose(0, 3, 1, 2))  # (T,CIN,128,128)
    if not flip:
        xs[:, :, 0:SLAB, 0:128] = xc[:, :, 0:SLAB, :]
    else:
        # slab[s] = x_global[128 - s]; s=0 is the zero pad row
        xs[:, :, 1:SLAB, 0:128] = xc[:, :, 128 - SLAB + 1:][:, :, ::-1, :]

    # stride-2 compact extraction: xbk[t, 32*(3j+d)+ch, q, w] =
    # slab[t, ch, 2q+j, 2w+d]
    qi = 2 * np.arange(RX)[:, None]
    wi = 2 * np.arange(W2)[None, :]
    xbk = np.zeros((T, 288, RX, W2), np.float32)
    for j in range(3):
        for dcol in range(3):
            g = 3 * j + dcol
            xbk[:, 32*g:32*g+32] = xs[:, :, qi + j, wi + dcol]

    Wk = W[::-1].copy() if flip else W
    Uk = U[::-1].copy() if flip else U

    ua = np.zeros((128, 768), np.float32)
    qaw = np.zeros((128, 256), np.float32)
    qbw = np.zeros((128, 256), np.float32)
    qcw = np.zeros((128, 256), np.float32)
    qdw = np.zeros((96, 256), np.float32)
    for di in range(3):
        for m in range(2):
            g = di * 2 + m
            cols = slice(g * 128, (g + 1) * 128)
            mc = slice(m * 128, (m + 1) * 128)
            ua[0:64, cols] = Uk[0, di, :, mc]
            ua[64:128, cols] = Uk[1, di, :, mc]
    for m in range(2):
        mc = slice(m * 128, (m + 1) * 128)
        mo = slice(m * 128, (m + 1) * 128)
        qaw[0:64, mo] = Uk[2, 0, :, mc]
        qaw[64:128, mo] = Uk[2, 1, :, mc]
        qbw[0:64, mo] = Uk[2, 2, :, mc]
        qbw[64:96, mo] = Wk[0, 0, :, mc]
        qbw[96:128, mo] = Wk[0, 1, :, mc]
        qcw[0:32, mo] = Wk[0, 2, :, mc]
        qcw[32:64, mo] = Wk[1, 0, :, mc]
        qcw[64:96, mo] = Wk[1, 1, :, mc]
        qcw[96:128, mo] = Wk[1, 2, :, mc]
        qdw[0:32, mo] = Wk[2, 0, :, mc]
        qdw[32:64, mo] = Wk[2, 1, :, mc]
        qdw[64:96, mo] = Wk[2, 2, :, mc]

    eps = 1e-3
    scale = (gamma / np.sqrt(moving_var + eps)).astype(np.float32)
    beta2 = (beta - moving_mean * scale).astype(np.float32)
    vecs = np.zeros((128, 8), np.float32)
    vecs[:, 0] = 0.2 * b[0:128] + 0.5
    vecs[0:64, 1] = b[128:192]
    vecs[64:128, 2] = 0.2 * b[192:256] + 0.5
    vecs[0:64, 3] = scale
    vecs[0:64, 4] = beta2
    return {
        "xbk": np.ascontiguousarray(
            xbk.reshape(T, 288, RX * W2).astype(sdt)),
        "ua": np.ascontiguousarray(ua.astype(sdt)),
        "qaw": np.ascontiguousarray(qaw.astype(sdt)),
        "qbw": np.ascontiguousarray(qbw.astype(sdt)),
        "qcw": np.ascontiguousarray(qcw.astype(sdt)),
        "qdw": np.ascontiguousarray(qdw.astype(sdt)),
        "vecs": vecs,
    }


def _patch_tile_drain():
    """This walrus build encodes at most ONE sync wait per CTRL instruction;
    split the Tile exit drain's waits across SP nops."""
    import bass_rust
    import concourse.tile as tile
    from concourse.vector_clock import ScopedClock
    if getattr(tile.TileContext, "_drain_patched", False):
        return

    def patched(self, tick_clock, wait_clock):
        drain_inst = self.nc.sync.drain()
        wait_clock.add_sem_waits(
            drain_inst.ins, ScopedClock({None: tick_clock.global_clock}))
        si = drain_inst.ins.sync_info
        waits = list(si.on_wait) if si is not None else []
        if len(waits) > 1:
            si.on_wait = waits[:1]
            for w in waits[1:]:
                nop = self.nc.sync.nop()
                nsi = nop.ins.sync_info
                if nsi is None:
                    nop.ins.sync_info = bass_rust.SyncInfo(
                        on_wait=[w], on_update=[])
                else:
                    nsi.on_wait = [w]
        self.nc.all_engine_barrier()
        assert self.sems is not None
        popped = self.nc._tile_sem_poison_stack.pop()
        assert popped is self._sem_poison
        self.nc.clear_and_free_semaphores(list(self.sems.allocated().values()))
        self.nc.all_engine_barrier()

    tile.TileContext._drain_and_barrier = patched
    tile.TileContext._drain_patched = True


def _split_multi_waits(nc, mybir):
    """This walrus build encodes at most one sync wait per instruction;
    move excess waits onto single-wait nops inserted just before."""
    ctr = 0
    for bb in nc.main_func.blocks:
        insts = bb.instructions
        out = []
        changed = False
        for inst in insts:
            si = inst.sync_info
            waits = list(si.on_wait) if si is not None else []
            if len(waits) > 1:
                changed = True
                for w in waits[:-1]:
                    ctr += 1
                    out.append(mybir.InstNoOp(
                        name=f"wsplit-{ctr}",
                        engine=inst.engine,
                        sync_info=mybir.SyncInfo(on_wait=[w], on_update=[]),
                        bass_nofuse=True))
                si.on_wait = [waits[-1]]
            out.append(inst)
        if changed:
            bb.instructions = out


def _build_nc():
    import concourse.bass as bass
    import concourse.mybir as mybir
    import concourse.tile as tile
    _patch_tile_drain()
    dt = mybir.dt
    sdt = dt.bfloat16 if MM_DT == "bf16" else dt.float32
    AF = mybir.ActivationFunctionType

    def mm_ap(ap):
        return ap.bitcast(dt.float32r) if MM_DT == "fp32r" else ap

    nc = bass.Bass()
    xbk = nc.dram_tensor("xbk", [T, 288, RX * W2], sdt, kind="ExternalInput")
    ua = nc.dram_tensor("ua", [128, 768], sdt, kind="ExternalInput")
    qaw = nc.dram_tensor("qaw", [128, 256], sdt, kind="ExternalInput")
    qbw = nc.dram_tensor("qbw", [128, 256], sdt, kind="ExternalInput")
    qcw = nc.dram_tensor("qcw", [128, 256], sdt, kind="ExternalInput")
    qdw = nc.dram_tensor("qdw", [96, 256], sdt, kind="ExternalInput")
    vecs = nc.dram_tensor("vecs", [128, 8], dt.float32, kind="ExternalInput")
    y = nc.dram_tensor("y", [T, F, R * W2], dt.float32, kind="ExternalOutput")

    groups = [[0, 1], [2, 3], [4, 5], [6, 7]]

    with tile.TileContext(nc) as tc:
        with (
            tc.tile_pool(name="const", bufs=1) as cpool,
            tc.tile_pool(name="state", bufs=1) as spool,
            tc.tile_pool(name="xp", bufs=2) as xpool,
            tc.tile_pool(name="ps", bufs=6, space="PSUM") as pspool,
            tc.tile_pool(name="psm", bufs=2, space="PSUM") as pmpool,
            tc.tile_pool(name="epi", bufs=3) as epool,
            tc.tile_pool(name="halo", bufs=2) as hpool,
            tc.tile_pool(name="dram", bufs=2, space="DRAM") as dpool,
        ):
            uasb = cpool.tile([128, 768], sdt, tag="uasb")
            qawsb = cpool.tile([128, 256], sdt, tag="qawsb")
            qbwsb = cpool.tile([128, 256], sdt, tag="qbwsb")
            qcwsb = cpool.tile([128, 256], sdt, tag="qcwsb")
            qdwsb = cpool.tile([96, 256], sdt, tag="qdwsb")
            vsb = cpool.tile([128, 8], dt.float32, tag="vsb")
            # spread the startup DMAs across queues so the first matmuls
            # aren't gated on one serial queue
            nc.scalar.dma_start(out=uasb[:], in_=ua[:])
            nc.scalar.dma_start(out=qawsb[:], in_=qaw[:])
            nc.scalar.dma_start(out=qbwsb[:], in_=qbw[:])
            nc.gpsimd.dma_start(out=qcwsb[:], in_=qcw[:])
            nc.gpsimd.dma_start(out=qdwsb[:], in_=qdw[:])
            nc.scalar.dma_start(out=vsb[:], in_=vecs[:])

            h2 = [spool.tile([128, HSLOT * WP], sdt, name=f"h2_{i}",
                             tag=f"h2_{i}")
                  for i in range(2)]
            # col-baked copies of h(row+1): qa2 = [col-1 ; col0],
            # qb2 parts 0:64 = col+1, parts 64:128 = x taps (j0,d0),(j0,d1)
            qa2 = [spool.tile([128, RX * W2], sdt, name=f"qa2_{i}",
                              tag=f"qa2_{i}") for i in range(2)]
            qb2 = [spool.tile([128, RX * W2], sdt, name=f"qb2_{i}",
                              tag=f"qb2_{i}") for i in range(2)]
            c_sb = spool.tile([64, (R + E) * W2], dt.float32, tag="c")
            nc.vector.memset(h2[0][:], 0.0)
            nc.vector.memset(h2[1][:], 0.0)
            nc.vector.memset(qa2[0][:], 0.0)
            nc.vector.memset(qb2[0][0:64, :], 0.0)
            nc.vector.memset(c_sb[:], 0.0)

            def conv_group(pss, xp2r, xp3r, qar, qbr, hpr, q0, nrow):
                """Accumulate the 7-pass conv group for rows q0..q0+nrow-1
                into PSUM tiles pss (one per gate-tile m)."""
                psrs = [ps[:].rearrange("p (a b) -> p a b", b=W2)[
                    :, 0:nrow, :] for ps in pss]
                for di in range(3):
                    d = di - 1
                    for m in range(2):
                        gcol = slice((di*2+m)*128, (di*2+m+1)*128)
                        nc.tensor.matmul(
                            psrs[m][:],
                            lhsT=mm_ap(uasb[0:128, gcol]),
                            rhs=mm_ap(hpr[0:128, q0:q0+nrow,
                                          1+d:65+d]),
                            start=(di == 0), stop=False)
                for m in range(2):
                    mo = slice(m * 128, (m + 1) * 128)
                    nc.tensor.matmul(
                        psrs[m][:], lhsT=mm_ap(qawsb[0:128, mo]),
                        rhs=mm_ap(qar[0:128, q0:q0+nrow, :]),
                        start=False, stop=False)
                for m in range(2):
                    mo = slice(m * 128, (m + 1) * 128)
                    nc.tensor.matmul(
                        psrs[m][:], lhsT=mm_ap(qbwsb[0:128, mo]),
                        rhs=mm_ap(qbr[0:128, q0:q0+nrow, :]),
                        start=False, stop=False)
                for m in range(2):
                    mo = slice(m * 128, (m + 1) * 128)
                    nc.tensor.matmul(
                        psrs[m][:], lhsT=mm_ap(qcwsb[0:128, mo]),
                        rhs=mm_ap(xp2r[0:128, q0:q0+nrow, :]),
                        start=False, stop=False)
                for m in range(2):
                    mo = slice(m * 128, (m + 1) * 128)
                    nc.tensor.matmul(
                        psrs[m][:], lhsT=mm_ap(qdwsb[0:96, mo]),
                        rhs=mm_ap(xp3r[0:96, q0:q0+nrow, :]),
                        start=False, stop=(m == 1))

            def epilogue(pss, hcr, qanr, qbnr, q0, nrow, store_y, t):
                """Gates + LSTM cell update for rows q0..q0+nrow-1; writes h
                (A, B-dup, and the three col-baked r+1 copies), c, and
                optionally y."""
                ps0, ps1 = pss
                psl = slice(0, nrow * W2)
                cs = slice(q0 * W2, (q0 + nrow) * W2)
                # per-gate ACTs, all landing at partition base 0
                # (2-input DVE ops require equal input base partitions)
                i_t = epool.tile([64, 512], dt.float32, tag="i")
                nc.scalar.activation(i_t[:, psl], ps0[0:64, psl], AF.Relu,
                                     bias=vsb[0:64, 0:1], scale=0.2)
                f_t = epool.tile([64, 512], dt.float32, tag="f")
                nc.scalar.activation(f_t[:, psl], ps0[64:128, psl],
                                     AF.Relu,
                                     bias=vsb[64:128, 0:1], scale=0.2)
                g_t = epool.tile([64, 512], dt.float32, tag="g")
                nc.scalar.activation(g_t[:, psl], ps1[0:64, psl], AF.Tanh,
                                     bias=vsb[0:64, 1:2], scale=1.0)
                o_t = epool.tile([64, 512], dt.float32, tag="o")
                nc.scalar.activation(o_t[:, psl], ps1[64:128, psl],
                                     AF.Relu,
                                     bias=vsb[64:128, 2:3], scale=0.2)
                # hard-sigmoid clip fused into the gate products:
                # t = (gate min 1.0) * other
                t1 = epool.tile([64, 512], dt.float32, tag="t1")
                nc.vector.scalar_tensor_tensor(
                    t1[:, psl], f_t[:, psl], 1.0, c_sb[:, cs],
                    mybir.AluOpType.min, mybir.AluOpType.mult)
                t2 = epool.tile([64, 512], dt.float32, tag="t2")
                nc.vector.scalar_tensor_tensor(
                    t2[:, psl], i_t[:, psl], 1.0, g_t[:, psl],
                    mybir.AluOpType.min, mybir.AluOpType.mult)
                nc.vector.tensor_add(c_sb[:, cs], t1[:, psl], t2[:, psl])
                tc_t = epool.tile([64, 512], dt.float32, tag="tc")
                nc.scalar.activation(tc_t[:, psl], c_sb[:, cs], AF.Tanh)
                hlo = hcr[0:64, q0+1:q0+nrow+1, 1:65]
                nc.vector.scalar_tensor_tensor(
                    hlo,
                    o_t[:, psl].rearrange("p (a b) -> p a b", b=W2), 1.0,
                    tc_t[:, psl].rearrange("p (a b) -> p a b", b=W2),
                    mybir.AluOpType.min, mybir.AluOpType.mult)
                nc.vector.tensor_copy(
                    out=hcr[64:128, q0:q0+nrow, 1:65], in_=hlo)
                # col-baked copies of rows q0..q0+nrow-1 into the r+1
                # tiles for the NEXT step (QA/QB slot s = row s+1 = A
                # slot s+2): one per engine queue to balance load.
                s0 = q0 - 1 if q0 > 0 else 0
                a0 = s0 + 2
                sn = q0 + nrow - 1
                nc.gpsimd.tensor_copy(
                    out=qanr[0:64, s0:sn, :],
                    in_=hcr[0:64, a0:sn+2, 0:64])
                nc.scalar.activation(
                    qanr[64:128, s0:sn, :],
                    hcr[0:64, a0:sn+2, 1:65], AF.Copy)
                nc.vector.tensor_copy(
                    out=qbnr[0:64, s0:sn, :],
                    in_=hcr[0:64, a0:sn+2, 2:66])
                if store_y:
                    yst = epool.tile([64, 512], dt.float32, tag="yst")
                    nc.gpsimd.tensor_scalar(
                        yst[:, psl].rearrange("p (a b) -> p a b", b=W2),
                        hlo,
                        vsb[0:64, 3:4], vsb[0:64, 4:5],
                        mybir.AluOpType.mult, mybir.AluOpType.add)
                    nc.sync.dma_start(out=y[t, :, q0*W2:(q0+nrow)*W2],
                                      in_=yst[:, psl])

            for t in range(T):
                j = t % E          # phase position; e = EXT - j extra rows
                e = EXT - j
                hc = h2[t % 2]
                hp = h2[(t + 1) % 2]
                hcr = hc[:].rearrange("p (q w) -> p q w", w=WP)
                hpr = hp[:].rearrange("p (q w) -> p q w", w=WP)
                # r+1 tiles: read current (filled during step t-1), write
                # next (consumed at step t+1)
                qar = qa2[t % 2][:].rearrange("p (q w) -> p q w", w=W2)
                qbr = qb2[t % 2][:].rearrange("p (q w) -> p q w", w=W2)
                qanr = qa2[(t + 1) % 2][:].rearrange(
                    "p (q w) -> p q w", w=W2)
                qbnr = qb2[(t + 1) % 2][:].rearrange(
                    "p (q w) -> p q w", w=W2)

                # x for step t: taps (j0,d0),(j0,d1) into qb parts 64:128,
                # the rest into xp2/xp3
                nc.sync.dma_start(out=qbr[64:128, :, :],
                                  in_=xbk[t, 0:64].rearrange(
                                      "p (q w) -> p q w", w=W2))
                xp2t = xpool.tile([128, RX * W2], sdt, tag="xp2")
                xp2r = xp2t[:].rearrange("p (q w) -> p q w", w=W2)
                nc.sync.dma_start(out=xp2r[:], in_=xbk[t, 64:192].rearrange(
                    "p (q w) -> p q w", w=W2))
                xp3t = xpool.tile([96, RX * W2], sdt, tag="xp3")
                xp3r = xp3t[:].rearrange("p (q w) -> p q w", w=W2)
                nc.sync.dma_start(out=xp3r[:], in_=xbk[t, 192:288].rearrange(
                    "p (q w) -> p q w", w=W2))

                exchange = (j == E - 1 and t < T - 1)
                # On exchange steps the boundary chunk runs first so the
                # AllReduce overlaps the rest of this step and the top
                # chunks of the next one; otherwise top-down order keeps
                # each chunk a full step ahead of its consumers.
                order = (3, 0, 1, 2) if exchange else (0, 1, 2, 3)

                for ci in order:
                    q0 = ci * NQ
                    pss = [pspool.tile([128, 512], dt.float32,
                                       name=f"ps_{t}_{ci}_{mi}", tag="ps")
                           for mi in range(2)]
                    conv_group(pss, xp2r, xp3r, qar, qbr, hpr, q0, NQ)
                    epilogue(pss, hcr, qanr, qbnr, q0, NQ, True, t)

                    if ci == 3 and exchange:
                        # pack own boundary rows 28..31 of (h, c) row-
                        # reversed (the flipped peer consumes them in its
                        # own orientation), AllReduce with the paired core,
                        # subtract own contribution, scatter the peer rows
                        # into the halo slots.
                        bsend = hpool.tile([64, 512], dt.float32,
                                           tag="bsend")
                        for k in range(E):
                            sl = R - k  # h A-slot of row 31-k
                            nc.gpsimd.tensor_copy(
                                out=bsend[:, k*64:(k+1)*64],
                                in_=hcr[0:64, sl, 1:65])
                            nc.gpsimd.tensor_copy(
                                out=bsend[:, 256+k*64:256+(k+1)*64],
                                in_=c_sb[:, (R-1-k)*W2:(R-k)*W2])
                        bin_d = dpool.tile([64, 512], dt.float32,
                                           tag="bin")
                        bout_d = dpool.tile([64, 512], dt.float32,
                                            tag="bout")
                        nc.sync.dma_start(out=bin_d[:], in_=bsend[:])
                        nc.gpsimd.collective_compute(
                            "AllReduce", mybir.AluOpType.add,
                            replica_groups=groups,
                            ins=[bin_d[:].opt()], outs=[bout_d[:].opt()])
                        bsum = hpool.tile([64, 512], dt.float32,
                                          tag="bsum")
                        nc.sync.dma_start(out=bsum[:], in_=bout_d[:])
                        # receive on the (otherwise idle) gpsimd queue so
                        # it runs the moment the collective lands.
                        recv = hpool.tile([64, 512], dt.float32,
                                          tag="recv")
                        nc.gpsimd.tensor_sub(recv[:], bsum[:], bsend[:])
                        rh = recv[:, 0:256].rearrange(
                            "p (a b) -> p a b", b=64)
                        nc.gpsimd.tensor_copy(
                            out=hcr[0:64, R+1:R+1+E, 1:65], in_=rh)
                        nc.gpsimd.tensor_copy(
                            out=hcr[64:128, R:R+E, 1:65], in_=rh)
                        nc.gpsimd.tensor_copy(
                            out=c_sb[:, R*W2:(R+E)*W2],
                            in_=recv[:, 256:512])
                        # halo rows into the r+1 col-baked tiles (slots
                        # 31..34 = rows 32..35 = A slots 33..36)
                        nc.gpsimd.tensor_copy(
                            out=qanr[0:64, R-1:R+EXT, :],
                            in_=hcr[0:64, R+1:R+1+E, 0:64])
                        nc.gpsimd.tensor_copy(
                            out=qanr[64:128, R-1:R+EXT, :],
                            in_=hcr[0:64, R+1:R+1+E, 1:65])
                        nc.gpsimd.tensor_copy(
                            out=qbnr[0:64, R-1:R+EXT, :],
                            in_=hcr[0:64, R+1:R+1+E, 2:66])

                if e > 0:
                    # mini chunk: redundant neighbor rows 32..31+e keep the
                    # wedge alive between halo exchanges. No y store.
                    q0 = R
                    pss = [pmpool.tile([128, EXT * W2], dt.float32,
                                       name=f"psm_{t}_{mi}", tag="psm")
                           for mi in range(2)]
                    conv_group(pss, xp2r, xp3r, qar, qbr, hpr, q0, e)
                    epilogue(pss, hcr, qanr, qbnr, q0, e, False, t)
    _split_multi_waits(nc, mybir)
    return nc


def _install_ntff_hook():
    """The image's antenv lacks axon_hooks; synthesize it and register the
    ctypes NTFF profile hook so trace=True works under axon."""
    import sys
    import types
    try:
        from antenv.axon_hooks import get_axon_ntff_profile_hook  # noqa
        return
    except ImportError:
        pass
    mod = types.ModuleType("antenv.axon_hooks")
    mod._hook = None

    def set_axon_ntff_profile_hook(h):
        mod._hook = h

    def get_axon_ntff_profile_hook():
        return mod._hook

    mod.set_axon_ntff_profile_hook = set_axon_ntff_profile_hook
    mod.get_axon_ntff_profile_hook = get_axon_ntff_profile_hook
    sys.modules["antenv.axon_hooks"] = mod
    import antenv
    antenv.axon_hooks = mod
    try:
        from trn_agent_boot.trn_boot import _ntff_profile_via_ctypes
        hook = _ntff_profile_via_ctypes("/opt/axon/libaxon_pjrt.so")
        if hook is not None:
            mod._hook = hook
    except Exception:
        pass


def _get_nc():
    key = (MM_DT,)
    if key not in _CACHE:
        _CACHE[key] = _build_nc()
    return _CACHE[key]


def kernel(x, W, U, b, gamma, beta, moving_mean, moving_var):
    from concourse.bass_utils import run_bass_kernel_spmd
    x = np.asarray(x, np.float32)
    W = np.asarray(W, np.float32)
    U = np.asarray(U, np.float32)
    b = np.asarray(b, np.float32)
    gamma = np.asarray(gamma, np.float32)
    beta = np.asarray(beta, np.float32)
    moving_mean = np.asarray(moving_mean, np.float32)
    moving_var = np.asarray(moving_var, np.float32)
    B = x.shape[0]

    in_maps = []
    for bidx in range(B):
        for half in range(N_HALVES):
            in_maps.append(_prep_core_inputs(
                x, W, U, b, gamma, beta, moving_mean, moving_var, bidx, half))

    nc = _get_nc()
    trace = os.environ.get("BASS_KERNEL_TRACE") == "1"
    if trace:
        _install_ntff_hook()
    res = run_bass_kernel_spmd(nc, in_maps, core_ids=list(range(8)),
                               trace=trace)
    kernel._last_result = res

    out = np.zeros((B, T, H2, W2, F), np.float32)
    ci = 0
    for bidx in range(B):
        for half in range(N_HALVES):
            yc = res.results[ci]["y"].reshape(T, F, R, W2)
            ci += 1
            yc = yc.transpose(0, 2, 3, 1)  # (T, R, W2, F)
            if half == 1:
                yc = yc[:, ::-1, :, :]
                out[bidx, :, 32:64] = yc
            else:
                out[bidx, :, 0:32] = yc
    return out


# revision 11
# speedup vs baseline: 1.4802x; 1.4802x over previous
"""ConvLSTM block Trainium2 kernel (8 NeuronCores).

Sharding: 8 cores = 4 batches x 2 H-halves. Bottom-half cores process their
slab vertically flipped (with kh-flipped conv kernels) so one SPMD program
serves all cores.

Halo scheme: instead of exchanging one boundary row of h every step (15
AllReduces of ~8-16us latency each on the critical path), the cores exchange
a 4-row halo of (h, c) every 4 steps (after steps 3, 7, 11 -> 3 AllReduces).
Between exchanges each core redundantly computes a shrinking wedge of the
neighbor's rows (3, 2, 1, 0 extra rows per step within a phase) in a small
"mini" PSUM group, keeping all owned rows exact at every step.

Per-core compute per step: for each of 2 output-channel tiles (128 ch) and
each chunk of 8 output rows (N=512 pixels), one PSUM accumulation group of
9 matmuls: 3x input conv (K=96: 3 row-taps x 32ch; stride-2 column access
via strided APs) + 6x recurrent conv (K=128: 2 row-taps x 64ch using a
row-shifted duplicate copy of h in partitions 64:128). Gates/LSTM/BN run on
ACT+DVE out of PSUM; bias and BN are folded into the activations.

Scheduling: on exchange steps the boundary chunk (rows 24..31) runs first so
the AllReduce is in flight ~5us into the step; its consumers (next step's
boundary chunk + mini) run ~23us later, hiding the collective. On other
steps chunks run top-down (0,1,2,3,mini) so each chunk's rows are produced
a full step before the next step's consumer chunk reads them. The mini
group lives in its own 2-bank PSUM pool so its reuse cannot serialize the
main 6-bank chunk ring. Halo receive (sub + copies) and the BN affine run
on the otherwise-idle GpSimd queue.
"""
import os
import numpy as np

T, H2, W2, F, CIN = 16, 64, 64, 64, 32
WP, XW, NQ = 66, 130, 8
N_HALVES = 2
R = H2 // N_HALVES
E = 4                 # halo width / steps per exchange phase
EXT = E - 1           # max redundant rows per step
RX = R + EXT          # rows of conv output prepared per step
SLAB = 2 * RX + 1     # input rows needed per slab
HSLOT = R + 6         # h row-slot count (pad + R owned + ext/halo + pad)
NCHUNK = R // NQ
MM_DT = os.environ.get("CONV_LSTM_MM_DT", "bf16")  # bf16 | fp32 | fp32r

_CACHE = {}


def _storage_np_dtype():
    import ml_dtypes
    return ml_dtypes.bfloat16 if MM_DT == "bf16" else np.float32


def _prep_core_inputs(x, W, U, b, gamma, beta, moving_mean, moving_var,
                      bidx, half):
    sdt = _storage_np_dtype()
    flip = (half == 1)

    # x slab [T, CIN, SLAB, XW]; XLA SAME (stride2,k3,even) pads bottom/right
    # only: out row r reads input rows 2r..2r+2 (row/col 128 = zero pad).
    xs = np.zeros((T, CIN, SLAB, XW), np.float32)
    xc = np.ascontiguousarray(x[bidx].transpose(0, 3, 1, 2))  # (T,CIN,128,128)
    if not flip:
        xs[:, :, 0:SLAB, 0:128] = xc[:, :, 0:SLAB, :]
    else:
        # slab[s] = x_global[128 - s]; s=0 is the zero pad row
        xs[:, :, 1:SLAB, 0:128] = xc[:, :, 128 - SLAB + 1:][:, :, ::-1, :]

    Wk = W[::-1].copy() if flip else W
    Uk = U[::-1].copy() if flip else U

    w3 = np.zeros((96, 768), np.float32)
    ua = np.zeros((128, 768), np.float32)
    ub = np.zeros((128, 768), np.float32)
    for di in range(3):
        for m in range(2):
            g = di * 2 + m
            cols = slice(g * 128, (g + 1) * 128)
            mc = slice(m * 128, (m + 1) * 128)
            for j in range(3):
                w3[32*j:32*j+32, cols] = Wk[j, di, :, mc]
            ua[0:64, cols] = Uk[0, di, :, mc]
            ua[64:128, cols] = Uk[1, di, :, mc]
            ub[0:64, cols] = Uk[2, di, :, mc]

    eps = 1e-3
    scale = (gamma / np.sqrt(moving_var + eps)).astype(np.float32)
    beta2 = (beta - moving_mean * scale).astype(np.float32)
    vecs = np.zeros((128, 8), np.float32)
    vecs[:, 0] = 0.2 * b[0:128] + 0.5
    vecs[0:64, 1] = b[128:192]
    vecs[64:128, 2] = 0.2 * b[192:256] + 0.5
    vecs[0:64, 3] = scale
    vecs[0:64, 4] = beta2
    return {
        "xs": np.ascontiguousarray(xs.astype(sdt)),
        "w3": np.ascontiguousarray(w3.astype(sdt)),
        "ua": np.ascontiguousarray(ua.astype(sdt)),
        "ub": np.ascontiguousarray(ub.astype(sdt)),
        "vecs": vecs,
    }


def _patch_tile_drain():
    """This walrus build encodes at most ONE sync wait per CTRL instruction;
    split the Tile exit drain's waits across SP nops."""
    import bass_rust
    import concourse.tile as tile
    from concourse.vector_clock import ScopedClock
    if getattr(tile.TileContext, "_drain_patched", False):
        return

    def patched(self, tick_clock, wait_clock):
        drain_inst = self.nc.sync.drain()
        wait_clock.add_sem_waits(
            drain_inst.ins, ScopedClock({None: tick_clock.global_clock}))
        si = drain_inst.ins.sync_info
        waits = list(si.on_wait) if si is not None else []
        if len(waits) > 1:
            si.on_wait = waits[:1]
            for w in waits[1:]:
                nop = self.nc.sync.nop()
                nsi = nop.ins.sync_info
                if nsi is None:
                    nop.ins.sync_info = bass_rust.SyncInfo(
                        on_wait=[w], on_update=[])
                else:
                    nsi.on_wait = [w]
        self.nc.all_engine_barrier()
        assert self.sems is not None
        popped = self.nc._tile_sem_poison_stack.pop()
        assert popped is self._sem_poison
        self.nc.clear_and_free_semaphores(list(self.sems.allocated().values()))
        self.nc.all_engine_barrier()

    tile.TileContext._drain_and_barrier = patched
    tile.TileContext._drain_patched = True


def _split_multi_waits(nc, mybir):
    """This walrus build encodes at most one sync wait per instruction;
    move excess waits onto single-wait nops inserted just before."""
    ctr = 0
    for bb in nc.main_func.blocks:
        insts = bb.instructions
        out = []
        changed = False
        for inst in insts:
            si = inst.sync_info
            waits = list(si.on_wait) if si is not None else []
            if len(waits) > 1:
                changed = True
                for w in waits[:-1]:
                    ctr += 1
                    out.append(mybir.InstNoOp(
                        name=f"wsplit-{ctr}",
                        engine=inst.engine,
                        sync_info=mybir.SyncInfo(on_wait=[w], on_update=[]),
                        bass_nofuse=True))
                si.on_wait = [waits[-1]]
            out.append(inst)
        if changed:
            bb.instructions = out


def _build_nc():
    import concourse.bass as bass
    import concourse.mybir as mybir
    import concourse.tile as tile
    _patch_tile_drain()
    dt = mybir.dt
    sdt = dt.bfloat16 if MM_DT == "bf16" else dt.float32
    AF = mybir.ActivationFunctionType

    def mm_ap(ap):
        return ap.bitcast(dt.float32r) if MM_DT == "fp32r" else ap

    nc = bass.Bass()
    xs = nc.dram_tensor("xs", [T, CIN, SLAB, XW], sdt, kind="ExternalInput")
    w3 = nc.dram_tensor("w3", [96, 768], sdt, kind="ExternalInput")
    ua = nc.dram_tensor("ua", [128, 768], sdt, kind="ExternalInput")
    ub = nc.dram_tensor("ub", [128, 768], sdt, kind="ExternalInput")
    vecs = nc.dram_tensor("vecs", [128, 8], dt.float32, kind="ExternalInput")
    y = nc.dram_tensor("y", [T, F, R * W2], dt.float32, kind="ExternalOutput")

    groups = [[0, 1], [2, 3], [4, 5], [6, 7]]

    with tile.TileContext(nc) as tc:
        with (
            tc.tile_pool(name="const", bufs=1) as cpool,
            tc.tile_pool(name="state", bufs=1) as spool,
            tc.tile_pool(name="xp", bufs=2) as xpool,
            tc.tile_pool(name="ps", bufs=6, space="PSUM") as pspool,
            tc.tile_pool(name="psm", bufs=2, space="PSUM") as pmpool,
            tc.tile_pool(name="epi", bufs=3) as epool,
            tc.tile_pool(name="halo", bufs=2) as hpool,
            tc.tile_pool(name="dram", bufs=2, space="DRAM") as dpool,
        ):
            w3sb = cpool.tile([96, 768], sdt, tag="w3sb")
            uasb = cpool.tile([128, 768], sdt, tag="uasb")
            ubsb = cpool.tile([128, 768], sdt, tag="ubsb")
            vsb = cpool.tile([128, 8], dt.float32, tag="vsb")
            # keep the sync queue free for the x loads: weights ride the
            # scalar/gpsimd queues so the first matmul isn't serialized
            # behind them
            nc.scalar.dma_start(out=w3sb[:], in_=w3[:])
            nc.scalar.dma_start(out=uasb[:], in_=ua[:])
            nc.gpsimd.dma_start(out=ubsb[:], in_=ub[:])
            nc.gpsimd.dma_start(out=vsb[:], in_=vecs[:])

            h2 = [spool.tile([128, HSLOT * WP], sdt, name=f"h2_{i}",
                             tag=f"h2_{i}")
                  for i in range(2)]
            # c lives on partitions 64:128 so the f-gate half of the merged
            # i/f activation (also at base 64) can multiply it directly
            c2 = spool.tile([128, (R + E) * W2], dt.float32, tag="c")
            nc.vector.memset(h2[0][:], 0.0)
            nc.vector.memset(h2[1][:], 0.0)
            nc.vector.memset(c2[:], 0.0)

            def conv_group(pss, x3r, hpr, q0, nrow):
                """Accumulate the 9-matmul conv group for rows q0..q0+nrow-1
                into PSUM tiles pss (one per gate-tile m)."""
                psrs = [ps[:].rearrange("p (a b) -> p a b", b=W2)[
                    :, 0:nrow, :] for ps in pss]
                for di in range(3):
                    d = di - 1
                    for m in range(2):
                        gcol = slice((di*2+m)*128, (di*2+m+1)*128)
                        nc.tensor.matmul(
                            psrs[m][:],
                            lhsT=mm_ap(w3sb[0:96, gcol]),
                            rhs=mm_ap(x3r[0:96, q0:q0+nrow,
                                          d+1:d+129:2]),
                            start=(di == 0), stop=False)
                for di in range(3):
                    d = di - 1
                    for m in range(2):
                        gcol = slice((di*2+m)*128, (di*2+m+1)*128)
                        nc.tensor.matmul(
                            psrs[m][:],
                            lhsT=mm_ap(uasb[0:128, gcol]),
                            rhs=mm_ap(hpr[0:128, q0:q0+nrow,
                                          1+d:65+d]),
                            start=False, stop=False)
                for di in range(3):
                    d = di - 1
                    for m in range(2):
                        gcol = slice((di*2+m)*128, (di*2+m+1)*128)
                        nc.tensor.matmul(
                            psrs[m][:],
                            lhsT=mm_ap(ubsb[0:128, gcol]),
                            rhs=mm_ap(hpr[0:128, q0+2:q0+nrow+2,
                                          1+d:65+d]),
                            start=False, stop=(di == 2))

            def epilogue(pss, hcr, q0, nrow, store_y, t):
                """Gates + LSTM cell update for rows q0..q0+nrow-1; writes h
                (both copies), c, and optionally y."""
                ps0, ps1 = pss
                psl = slice(0, nrow * W2)
                cs = slice(q0 * W2, (q0 + nrow) * W2)
                # i and f share the hard-sigmoid affine, so one 128-part
                # ACT covers both (i at 0:64, f at 64:128 like the PSUM
                # layout); c sits at base 64 so f*c has equal input bases.
                if_t = epool.tile([128, 512], dt.float32, tag="if")
                nc.scalar.activation(if_t[:, psl], ps0[0:128, psl], AF.Relu,
                                     bias=vsb[0:128, 0:1], scale=0.2)
                g_t = epool.tile([64, 512], dt.float32, tag="g")
                nc.scalar.activation(g_t[:, psl], ps1[0:64, psl], AF.Tanh,
                                     bias=vsb[0:64, 1:2], scale=1.0)
                o_t = epool.tile([64, 512], dt.float32, tag="o")
                nc.scalar.activation(o_t[:, psl], ps1[64:128, psl],
                                     AF.Relu,
                                     bias=vsb[64:128, 2:3], scale=0.2)
                # hard-sigmoid clip fused into the gate products:
                # t = (gate min 1.0) * other
                t1 = epool.tile([64, 512], dt.float32, tag="t1")
                nc.vector.scalar_tensor_tensor(
                    t1[:, psl], if_t[64:128, psl], 1.0, c2[64:128, cs],
                    mybir.AluOpType.min, mybir.AluOpType.mult)
                t2 = epool.tile([64, 512], dt.float32, tag="t2")
                nc.vector.scalar_tensor_tensor(
                    t2[:, psl], if_t[0:64, psl], 1.0, g_t[:, psl],
                    mybir.AluOpType.min, mybir.AluOpType.mult)
                nc.vector.tensor_add(c2[64:128, cs], t1[:, psl], t2[:, psl])
                tc_t = epool.tile([64, 512], dt.float32, tag="tc")
                nc.scalar.activation(tc_t[:, psl], c2[64:128, cs], AF.Tanh)
                hlo = hcr[0:64, q0+1:q0+nrow+1, 1:65]
                nc.vector.scalar_tensor_tensor(
                    hlo,
                    o_t[:, psl].rearrange("p (a b) -> p a b", b=W2), 1.0,
                    tc_t[:, psl].rearrange("p (a b) -> p a b", b=W2),
                    mybir.AluOpType.min, mybir.AluOpType.mult)
                nc.vector.tensor_copy(
                    out=hcr[64:128, q0:q0+nrow, 1:65], in_=hlo)
                if store_y:
                    yst = epool.tile([64, 512], dt.float32, tag="yst")
                    nc.gpsimd.tensor_scalar(
                        yst[:, psl].rearrange("p (a b) -> p a b", b=W2),
                        hlo,
                        vsb[0:64, 3:4], vsb[0:64, 4:5],
                        mybir.AluOpType.mult, mybir.AluOpType.add)
                    nc.sync.dma_start(out=y[t, :, q0*W2:(q0+nrow)*W2],
                                      in_=yst[:, psl])

            def load_x(tt):
                """Issue the x row-parity loads for step tt (sync queue)."""
                x3t = xpool.tile([96, RX * XW], sdt, tag="x3",
                                 name=f"x3_{tt}")
                x3r = x3t[:].rearrange("p (q w) -> p q w", w=XW)
                nc.sync.dma_start(out=x3r[0:32], in_=xs[tt, :, 0:2*RX-1:2, :])
                nc.sync.dma_start(out=x3r[32:64], in_=xs[tt, :, 1:2*RX:2, :])
                nc.sync.dma_start(out=x3r[64:96], in_=xs[tt, :, 2:2*RX+1:2, :])
                return x3r

            x3_next = load_x(0)
            for t in range(T):
                j = t % E          # phase position; e = EXT - j extra rows
                e = EXT - j
                hc = h2[t % 2]
                hp = h2[(t + 1) % 2]
                hcr = hc[:].rearrange("p (q w) -> p q w", w=WP)
                hpr = hp[:].rearrange("p (q w) -> p q w", w=WP)

                # prefetch next step's x a full step ahead so its trigger
                # latency (behind this step's y stores on the sync queue)
                # never gates a matmul
                x3r = x3_next
                if t + 1 < T:
                    x3_next = load_x(t + 1)

                exchange = (j == E - 1 and t < T - 1)
                # On exchange steps the boundary chunk runs first so the
                # AllReduce overlaps the rest of this step and the top
                # chunks of the next one; otherwise top-down order keeps
                # each chunk a full step ahead of its consumers.
                order = (3, 0, 1, 2) if exchange else (0, 1, 2, 3)

                for ci in order:
                    q0 = ci * NQ
                    pss = [pspool.tile([128, 512], dt.float32,
                                       name=f"ps_{t}_{ci}_{mi}", tag="ps")
                           for mi in range(2)]
                    conv_group(pss, x3r, hpr, q0, NQ)
                    epilogue(pss, hcr, q0, NQ, True, t)

                    if ci == 3 and exchange:
                        # pack own boundary rows 28..31 of (h, c) row-
                        # reversed (the flipped peer consumes them in its
                        # own orientation), AllReduce with the paired core,
                        # subtract own contribution, scatter the peer rows
                        # into the halo slots.
                        bsend = hpool.tile([64, 512], dt.float32,
                                           tag="bsend")
                        for k in range(E):
                            sl = R - k  # h A-slot of row 31-k
                            nc.gpsimd.tensor_copy(
                                out=bsend[:, k*64:(k+1)*64],
                                in_=hcr[0:64, sl, 1:65])
                            nc.gpsimd.tensor_copy(
                                out=bsend[:, 256+k*64:256+(k+1)*64],
                                in_=c2[64:128, (R-1-k)*W2:(R-k)*W2])
                        bin_d = dpool.tile([64, 512], dt.float32,
                                           tag="bin")
                        bout_d = dpool.tile([64, 512], dt.float32,
                                            tag="bout")
                        nc.gpsimd.dma_start(out=bin_d[:], in_=bsend[:])
                        nc.gpsimd.collective_compute(
                            "AllReduce", mybir.AluOpType.add,
                            replica_groups=groups,
                            ins=[bin_d[:].opt()], outs=[bout_d[:].opt()])
                        bsum = hpool.tile([64, 512], dt.float32,
                                          tag="bsum")
                        nc.gpsimd.dma_start(out=bsum[:], in_=bout_d[:])
                        # receive on the (otherwise idle) gpsimd queue so
                        # it runs the moment the collective lands.
                        recv = hpool.tile([64, 512], dt.float32,
                                          tag="recv")
                        nc.gpsimd.tensor_sub(recv[:], bsum[:], bsend[:])
                        rh = recv[:, 0:256].rearrange(
                            "p (a b) -> p a b", b=64)
                        nc.gpsimd.tensor_copy(
                            out=hcr[0:64, R+1:R+1+E, 1:65], in_=rh)
                        nc.gpsimd.tensor_copy(
                            out=hcr[64:128, R:R+E, 1:65], in_=rh)
                        nc.gpsimd.tensor_copy(
                            out=c2[64:128, R*W2:(R+E)*W2],
                            in_=recv[:, 256:512])

                if e > 0:
                    # mini chunk: redundant neighbor rows 32..31+e keep the
                    # wedge alive between halo exchanges. No y store.
                    q0 = R
                    pss = [pmpool.tile([128, EXT * W2], dt.float32,
                                       name=f"psm_{t}_{mi}", tag="psm")
                           for mi in range(2)]
                    conv_group(pss, x3r, hpr, q0, e)
                    epilogue(pss, hcr, q0, e, False, t)
    _split_multi_waits(nc, mybir)
    return nc


def _install_ntff_hook():
    """The image's antenv lacks axon_hooks; synthesize it and register the
    ctypes NTFF profile hook so trace=True works under axon."""
    import sys
    import types
    try:
        from antenv.axon_hooks import get_axon_ntff_profile_hook  # noqa
        return
    except ImportError:
        pass
    mod = types.ModuleType("antenv.axon_hooks")
    mod._hook = None

    def set_axon_ntff_profile_hook(h):
        mod._hook = h

    def get_axon_ntff_profile_hook():
        return mod._hook

    mod.set_axon_ntff_profile_hook = set_axon_ntff_profile_hook
    mod.get_axon_ntff_profile_hook = get_axon_ntff_profile_hook
    sys.modules["antenv.axon_hooks"] = mod
    import antenv
    antenv.axon_hooks = mod
    try:
        from trn_agent_boot.trn_boot import _ntff_profile_via_ctypes
        hook = _ntff_profile_via_ctypes("/opt/axon/libaxon_pjrt.so")
        if hook is not None:
            mod._hook = hook
    except Exception:
        pass


def _get_nc():
    key = (MM_DT,)
    if key not in _CACHE:
        _CACHE[key] = _build_nc()
    return _CACHE[key]


def kernel(x, W, U, b, gamma, beta, moving_mean, moving_var):
    from concourse.bass_utils import run_bass_kernel_spmd
    x = np.asarray(x, np.float32)
    W = np.asarray(W, np.float32)
    U = np.asarray(U, np.float32)
    b = np.asarray(b, np.float32)
    gamma = np.asarray(gamma, np.float32)
    beta = np.asarray(beta, np.float32)
    moving_mean = np.asarray(moving_mean, np.float32)
    moving_var = np.asarray(moving_var, np.float32)
    B = x.shape[0]

    in_maps = []
    for bidx in range(B):
        for half in range(N_HALVES):
            in_maps.append(_prep_core_inputs(
                x, W, U, b, gamma, beta, moving_mean, moving_var, bidx, half))

    nc = _get_nc()
    trace = os.environ.get("BASS_KERNEL_TRACE") == "1"
    if trace:
        _install_ntff_hook()
    res = run_bass_kernel_spmd(nc, in_maps, core_ids=list(range(8)),
                               trace=trace)
    kernel._last_result = res

    out = np.zeros((B, T, H2, W2, F), np.float32)
    ci = 0
    for bidx in range(B):
        for half in range(N_HALVES):
            yc = res.results[ci]["y"].reshape(T, F, R, W2)
            ci += 1
            yc = yc.transpose(0, 2, 3, 1)  # (T, R, W2, F)
            if half == 1:
                yc = yc[:, ::-1, :, :]
                out[bidx, :, 32:64] = yc
            else:
                out[bidx, :, 0:32] = yc
    return out


# revision 12
# speedup vs baseline: 1.5489x; 1.0464x over previous
"""ConvLSTM block Trainium2 kernel (8 NeuronCores).

Sharding: 8 cores = 4 batches x 2 H-halves. Bottom-half cores process their
slab vertically flipped (with kh-flipped conv kernels) so one SPMD program
serves all cores.

Halo scheme: instead of exchanging one boundary row of h every step (15
AllReduces of ~8-16us latency each on the critical path), the cores exchange
a 4-row halo of (h, c) every 4 steps (after steps 3, 7, 11 -> 3 AllReduces).
Between exchanges each core redundantly computes a shrinking wedge of the
neighbor's rows (3, 2, 1, 0 extra rows per step within a phase) in a small
"mini" PSUM group, keeping all owned rows exact at every step.

Per-core compute per step: for each of 2 output-channel tiles (128 ch) and
each chunk of 8 output rows (N=512 pixels), one PSUM accumulation group of
9 matmuls: 3x input conv (K=96: 3 row-taps x 32ch; stride-2 column access
via strided APs) + 6x recurrent conv (K=128: 2 row-taps x 64ch using a
row-shifted duplicate copy of h in partitions 64:128). Gates/LSTM/BN run on
ACT+DVE out of PSUM; bias and BN are folded into the activations.

Scheduling: on exchange steps the boundary chunk (rows 24..31) runs first so
the AllReduce is in flight ~5us into the step; its consumers (next step's
boundary chunk + mini) run ~23us later, hiding the collective. On other
steps chunks run top-down (0,1,2,3,mini) so each chunk's rows are produced
a full step before the next step's consumer chunk reads them. The mini
group lives in its own 2-bank PSUM pool so its reuse cannot serialize the
main 6-bank chunk ring. Halo receive (sub + copies) and the BN affine run
on the otherwise-idle GpSimd queue.
"""
import os
import numpy as np

T, H2, W2, F, CIN = 16, 64, 64, 64, 32
WP, XW, NQ = 66, 130, 8
N_HALVES = 2
R = H2 // N_HALVES
E = 4                 # halo width / steps per exchange phase
EXT = E - 1           # max redundant rows per step
RX = R + EXT          # rows of conv output prepared per step
SLAB = 2 * RX + 1     # input rows needed per slab
HSLOT = R + 6         # h row-slot count (pad + R owned + ext/halo + pad)
NCHUNK = R // NQ
MM_DT = os.environ.get("CONV_LSTM_MM_DT", "bf16")  # bf16 | fp32 | fp32r

_CACHE = {}


def _storage_np_dtype():
    import ml_dtypes
    return ml_dtypes.bfloat16 if MM_DT == "bf16" else np.float32


def _prep_core_inputs(x, W, U, b, gamma, beta, moving_mean, moving_var,
                      bidx, half):
    sdt = _storage_np_dtype()
    flip = (half == 1)

    # x slab [T, CIN, SLAB, XW]; XLA SAME (stride2,k3,even) pads bottom/right
    # only: out row r reads input rows 2r..2r+2 (row/col 128 = zero pad).
    xs = np.zeros((T, CIN, SLAB, XW), np.float32)
    xc = np.ascontiguousarray(x[bidx].transpose(0, 3, 1, 2))  # (T,CIN,128,128)
    if not flip:
        xs[:, :, 0:SLAB, 0:128] = xc[:, :, 0:SLAB, :]
    else:
        # slab[s] = x_global[128 - s]; s=0 is the zero pad row
        xs[:, :, 1:SLAB, 0:128] = xc[:, :, 128 - SLAB + 1:][:, :, ::-1, :]

    Wk = W[::-1].copy() if flip else W
    Uk = U[::-1].copy() if flip else U

    w3 = np.zeros((96, 768), np.float32)
    ua = np.zeros((128, 768), np.float32)
    ub = np.zeros((128, 768), np.float32)
    for di in range(3):
        for m in range(2):
            g = di * 2 + m
            cols = slice(g * 128, (g + 1) * 128)
            mc = slice(m * 128, (m + 1) * 128)
            for j in range(3):
                w3[32*j:32*j+32, cols] = Wk[j, di, :, mc]
            ua[0:64, cols] = Uk[0, di, :, mc]
            ua[64:128, cols] = Uk[1, di, :, mc]
            ub[0:64, cols] = Uk[2, di, :, mc]

    eps = 1e-3
    scale = (gamma / np.sqrt(moving_var + eps)).astype(np.float32)
    beta2 = (beta - moving_mean * scale).astype(np.float32)
    vecs = np.zeros((128, 8), np.float32)
    vecs[:, 0] = 0.2 * b[0:128] + 0.5
    vecs[0:64, 1] = b[128:192]
    vecs[64:128, 2] = 0.2 * b[192:256] + 0.5
    vecs[0:64, 3] = scale
    vecs[0:64, 4] = beta2
    return {
        "xs": np.ascontiguousarray(xs.astype(sdt)),
        "w3": np.ascontiguousarray(w3.astype(sdt)),
        "ua": np.ascontiguousarray(ua.astype(sdt)),
        "ub": np.ascontiguousarray(ub.astype(sdt)),
        "vecs": vecs,
    }


def _patch_tile_drain():
    """This walrus build encodes at most ONE sync wait per CTRL instruction;
    split the Tile exit drain's waits across SP nops."""
    import bass_rust
    import concourse.tile as tile
    from concourse.vector_clock import ScopedClock
    if getattr(tile.TileContext, "_drain_patched", False):
        return

    def patched(self, tick_clock, wait_clock):
        drain_inst = self.nc.sync.drain()
        wait_clock.add_sem_waits(
            drain_inst.ins, ScopedClock({None: tick_clock.global_clock}))
        si = drain_inst.ins.sync_info
        waits = list(si.on_wait) if si is not None else []
        if len(waits) > 1:
            si.on_wait = waits[:1]
            for w in waits[1:]:
                nop = self.nc.sync.nop()
                nsi = nop.ins.sync_info
                if nsi is None:
                    nop.ins.sync_info = bass_rust.SyncInfo(
                        on_wait=[w], on_update=[])
                else:
                    nsi.on_wait = [w]
        self.nc.all_engine_barrier()
        assert self.sems is not None
        popped = self.nc._tile_sem_poison_stack.pop()
        assert popped is self._sem_poison
        self.nc.clear_and_free_semaphores(list(self.sems.allocated().values()))
        self.nc.all_engine_barrier()

    tile.TileContext._drain_and_barrier = patched
    tile.TileContext._drain_patched = True


def _split_multi_waits(nc, mybir):
    """This walrus build encodes at most one sync wait per instruction;
    move excess waits onto single-wait nops inserted just before."""
    ctr = 0
    for bb in nc.main_func.blocks:
        insts = bb.instructions
        out = []
        changed = False
        for inst in insts:
            si = inst.sync_info
            waits = list(si.on_wait) if si is not None else []
            if len(waits) > 1:
                changed = True
                for w in waits[:-1]:
                    ctr += 1
                    out.append(mybir.InstNoOp(
                        name=f"wsplit-{ctr}",
                        engine=inst.engine,
                        sync_info=mybir.SyncInfo(on_wait=[w], on_update=[]),
                        bass_nofuse=True))
                si.on_wait = [waits[-1]]
            out.append(inst)
        if changed:
            bb.instructions = out


def _build_nc():
    import concourse.bass as bass
    import concourse.mybir as mybir
    import concourse.tile as tile
    _patch_tile_drain()
    dt = mybir.dt
    sdt = dt.bfloat16 if MM_DT == "bf16" else dt.float32
    AF = mybir.ActivationFunctionType

    def mm_ap(ap):
        return ap.bitcast(dt.float32r) if MM_DT == "fp32r" else ap

    nc = bass.Bass()
    xs = nc.dram_tensor("xs", [T, CIN, SLAB, XW], sdt, kind="ExternalInput")
    w3 = nc.dram_tensor("w3", [96, 768], sdt, kind="ExternalInput")
    ua = nc.dram_tensor("ua", [128, 768], sdt, kind="ExternalInput")
    ub = nc.dram_tensor("ub", [128, 768], sdt, kind="ExternalInput")
    vecs = nc.dram_tensor("vecs", [128, 8], dt.float32, kind="ExternalInput")
    y = nc.dram_tensor("y", [T, F, R * W2], dt.float32, kind="ExternalOutput")

    groups = [[0, 1], [2, 3], [4, 5], [6, 7]]

    with tile.TileContext(nc) as tc:
        with (
            tc.tile_pool(name="const", bufs=1) as cpool,
            tc.tile_pool(name="state", bufs=1) as spool,
            tc.tile_pool(name="xp", bufs=3) as xpool,
            tc.tile_pool(name="ps", bufs=6, space="PSUM") as pspool,
            tc.tile_pool(name="psm", bufs=2, space="PSUM") as pmpool,
            tc.tile_pool(name="epi", bufs=3) as epool,
            tc.tile_pool(name="halo", bufs=2) as hpool,
            tc.tile_pool(name="dram", bufs=2, space="DRAM") as dpool,
        ):
            w3sb = cpool.tile([96, 768], sdt, tag="w3sb")
            uasb = cpool.tile([128, 768], sdt, tag="uasb")
            ubsb = cpool.tile([128, 768], sdt, tag="ubsb")
            vsb = cpool.tile([128, 8], dt.float32, tag="vsb")
            # keep the sync queue free for the x loads: weights ride the
            # scalar/gpsimd queues so the first matmul isn't serialized
            # behind them
            nc.scalar.dma_start(out=w3sb[:], in_=w3[:])
            nc.scalar.dma_start(out=uasb[:], in_=ua[:])
            nc.gpsimd.dma_start(out=ubsb[:], in_=ub[:])
            nc.gpsimd.dma_start(out=vsb[:], in_=vecs[:])

            h2 = [spool.tile([128, HSLOT * WP], sdt, name=f"h2_{i}",
                             tag=f"h2_{i}")
                  for i in range(2)]
            # c lives on partitions 64:128 so the f-gate half of the merged
            # i/f activation (also at base 64) can multiply it directly
            c2 = spool.tile([128, (R + E) * W2], dt.float32, tag="c")
            nc.vector.memset(h2[0][:], 0.0)
            nc.vector.memset(h2[1][:], 0.0)
            nc.vector.memset(c2[:], 0.0)

            def conv_group(pss, x3r, hpr, q0, nrow):
                """Accumulate the 9-matmul conv group for rows q0..q0+nrow-1
                into PSUM tiles pss (one per gate-tile m)."""
                psrs = [ps[:].rearrange("p (a b) -> p a b", b=W2)[
                    :, 0:nrow, :] for ps in pss]
                for di in range(3):
                    d = di - 1
                    for m in range(2):
                        gcol = slice((di*2+m)*128, (di*2+m+1)*128)
                        nc.tensor.matmul(
                            psrs[m][:],
                            lhsT=mm_ap(w3sb[0:96, gcol]),
                            rhs=mm_ap(x3r[0:96, q0:q0+nrow,
                                          d+1:d+129:2]),
                            start=(di == 0), stop=False)
                for di in range(3):
                    d = di - 1
                    for m in range(2):
                        gcol = slice((di*2+m)*128, (di*2+m+1)*128)
                        nc.tensor.matmul(
                            psrs[m][:],
                            lhsT=mm_ap(uasb[0:128, gcol]),
                            rhs=mm_ap(hpr[0:128, q0:q0+nrow,
                                          1+d:65+d]),
                            start=False, stop=False)
                for di in range(3):
                    d = di - 1
                    for m in range(2):
                        gcol = slice((di*2+m)*128, (di*2+m+1)*128)
                        nc.tensor.matmul(
                            psrs[m][:],
                            lhsT=mm_ap(ubsb[0:128, gcol]),
                            rhs=mm_ap(hpr[0:128, q0+2:q0+nrow+2,
                                          1+d:65+d]),
                            start=False, stop=(di == 2))

            def epilogue(pss, hcr, q0, nrow, store_y, t):
                """Gates + LSTM cell update for rows q0..q0+nrow-1; writes h
                (both copies), c, and optionally y."""
                ps0, ps1 = pss
                psl = slice(0, nrow * W2)
                cs = slice(q0 * W2, (q0 + nrow) * W2)
                # i and f share the hard-sigmoid affine, so one 128-part
                # ACT covers both (i at 0:64, f at 64:128 like the PSUM
                # layout); c sits at base 64 so f*c has equal input bases.
                if_t = epool.tile([128, 512], dt.float32, tag="if")
                nc.scalar.activation(if_t[:, psl], ps0[0:128, psl], AF.Relu,
                                     bias=vsb[0:128, 0:1], scale=0.2)
                g_t = epool.tile([64, 512], dt.float32, tag="g")
                nc.scalar.activation(g_t[:, psl], ps1[0:64, psl], AF.Tanh,
                                     bias=vsb[0:64, 1:2], scale=1.0)
                o_t = epool.tile([64, 512], dt.float32, tag="o")
                nc.scalar.activation(o_t[:, psl], ps1[64:128, psl],
                                     AF.Relu,
                                     bias=vsb[64:128, 2:3], scale=0.2)
                # hard-sigmoid clip fused into the gate products:
                # t = (gate min 1.0) * other
                t1 = epool.tile([64, 512], dt.float32, tag="t1")
                nc.vector.scalar_tensor_tensor(
                    t1[:, psl], if_t[64:128, psl], 1.0, c2[64:128, cs],
                    mybir.AluOpType.min, mybir.AluOpType.mult)
                t2 = epool.tile([64, 512], dt.float32, tag="t2")
                nc.vector.scalar_tensor_tensor(
                    t2[:, psl], if_t[0:64, psl], 1.0, g_t[:, psl],
                    mybir.AluOpType.min, mybir.AluOpType.mult)
                nc.vector.tensor_add(c2[64:128, cs], t1[:, psl], t2[:, psl])
                tc_t = epool.tile([64, 512], dt.float32, tag="tc")
                nc.scalar.activation(tc_t[:, psl], c2[64:128, cs], AF.Tanh)
                hlo = hcr[0:64, q0+1:q0+nrow+1, 1:65]
                nc.vector.scalar_tensor_tensor(
                    hlo,
                    o_t[:, psl].rearrange("p (a b) -> p a b", b=W2), 1.0,
                    tc_t[:, psl].rearrange("p (a b) -> p a b", b=W2),
                    mybir.AluOpType.min, mybir.AluOpType.mult)
                nc.vector.tensor_copy(
                    out=hcr[64:128, q0:q0+nrow, 1:65], in_=hlo)
                if store_y:
                    yst = epool.tile([64, 512], dt.float32, tag="yst")
                    nc.gpsimd.tensor_scalar(
                        yst[:, psl].rearrange("p (a b) -> p a b", b=W2),
                        hlo,
                        vsb[0:64, 3:4], vsb[0:64, 4:5],
                        mybir.AluOpType.mult, mybir.AluOpType.add)
                    nc.sync.dma_start(out=y[t, :, q0*W2:(q0+nrow)*W2],
                                      in_=yst[:, psl])

            def load_x(tt, q=None):
                """Issue the x row-parity loads for step tt. Default queue
                is scalar: the sync queue carries the y stores, which can
                transiently block behind the halo collective - x must never
                sit behind them."""
                q = q or nc.scalar
                x3t = xpool.tile([96, RX * XW], sdt, tag="x3",
                                 name=f"x3_{tt}")
                x3r = x3t[:].rearrange("p (q w) -> p q w", w=XW)
                q.dma_start(out=x3r[0:32], in_=xs[tt, :, 0:2*RX-1:2, :])
                q.dma_start(out=x3r[32:64], in_=xs[tt, :, 1:2*RX:2, :])
                q.dma_start(out=x3r[64:96], in_=xs[tt, :, 2:2*RX+1:2, :])
                return x3r

            xbufs = {0: load_x(0, nc.sync), 1: load_x(1)}
            for t in range(T):
                j = t % E          # phase position; e = EXT - j extra rows
                e = EXT - j
                hc = h2[t % 2]
                hp = h2[(t + 1) % 2]
                hcr = hc[:].rearrange("p (q w) -> p q w", w=WP)
                hpr = hp[:].rearrange("p (q w) -> p q w", w=WP)

                # prefetch two steps ahead (ring of 3) so trigger latency
                # never gates a matmul
                x3r = xbufs.pop(t)
                if t + 2 < T:
                    xbufs[t + 2] = load_x(t + 2)

                exchange = (j == E - 1 and t < T - 1)
                # On exchange steps the boundary chunk runs first so the
                # AllReduce overlaps the rest of this step and the top
                # chunks of the next one; otherwise top-down order keeps
                # each chunk a full step ahead of its consumers.
                order = (3, 0, 1, 2) if exchange else (0, 1, 2, 3)

                for ci in order:
                    q0 = ci * NQ
                    pss = [pspool.tile([128, 512], dt.float32,
                                       name=f"ps_{t}_{ci}_{mi}", tag="ps")
                           for mi in range(2)]
                    conv_group(pss, x3r, hpr, q0, NQ)
                    epilogue(pss, hcr, q0, NQ, True, t)

                    if ci == 3 and exchange:
                        # pack own boundary rows 28..31 of (h, c) row-
                        # reversed (the flipped peer consumes them in its
                        # own orientation), AllReduce with the paired core,
                        # subtract own contribution, scatter the peer rows
                        # into the halo slots.
                        bsend = hpool.tile([64, 512], dt.float32,
                                           tag="bsend")
                        for k in range(E):
                            sl = R - k  # h A-slot of row 31-k
                            nc.vector.tensor_copy(
                                out=bsend[:, k*64:(k+1)*64],
                                in_=hcr[0:64, sl, 1:65])
                            nc.vector.tensor_copy(
                                out=bsend[:, 256+k*64:256+(k+1)*64],
                                in_=c2[64:128, (R-1-k)*W2:(R-k)*W2])
                        bin_d = dpool.tile([64, 512], dt.float32,
                                           tag="bin")
                        bout_d = dpool.tile([64, 512], dt.float32,
                                            tag="bout")
                        nc.gpsimd.dma_start(out=bin_d[:], in_=bsend[:])
                        nc.gpsimd.collective_compute(
                            "AllReduce", mybir.AluOpType.add,
                            replica_groups=groups,
                            ins=[bin_d[:].opt()], outs=[bout_d[:].opt()])
                        bsum = hpool.tile([64, 512], dt.float32,
                                          tag="bsum")
                        nc.gpsimd.dma_start(out=bsum[:], in_=bout_d[:])
                        # receive on the (otherwise idle) gpsimd queue so
                        # it runs the moment the collective lands.
                        recv = hpool.tile([64, 512], dt.float32,
                                          tag="recv")
                        nc.gpsimd.tensor_sub(recv[:], bsum[:], bsend[:])
                        rh = recv[:, 0:256].rearrange(
                            "p (a b) -> p a b", b=64)
                        nc.gpsimd.tensor_copy(
                            out=hcr[0:64, R+1:R+1+E, 1:65], in_=rh)
                        nc.gpsimd.tensor_copy(
                            out=hcr[64:128, R:R+E, 1:65], in_=rh)
                        nc.gpsimd.tensor_copy(
                            out=c2[64:128, R*W2:(R+E)*W2],
                            in_=recv[:, 256:512])

                if e > 0:
                    # mini chunk: redundant neighbor rows 32..31+e keep the
                    # wedge alive between halo exchanges. No y store.
                    q0 = R
                    pss = [pmpool.tile([128, EXT * W2], dt.float32,
                                       name=f"psm_{t}_{mi}", tag="psm")
                           for mi in range(2)]
                    conv_group(pss, x3r, hpr, q0, e)
                    epilogue(pss, hcr, q0, e, False, t)
    _split_multi_waits(nc, mybir)
    return nc


def _install_ntff_hook():
    """The image's antenv lacks axon_hooks; synthesize it and register the
    ctypes NTFF profile hook so trace=True works under axon."""
    import sys
    import types
    try:
        from antenv.axon_hooks import get_axon_ntff_profile_hook  # noqa
        return
    except ImportError:
        pass
    mod = types.ModuleType("antenv.axon_hooks")
    mod._hook = None

    def set_axon_ntff_profile_hook(h):
        mod._hook = h

    def get_axon_ntff_profile_hook():
        return mod._hook

    mod.set_axon_ntff_profile_hook = set_axon_ntff_profile_hook
    mod.get_axon_ntff_profile_hook = get_axon_ntff_profile_hook
    sys.modules["antenv.axon_hooks"] = mod
    import antenv
    antenv.axon_hooks = mod
    try:
        from trn_agent_boot.trn_boot import _ntff_profile_via_ctypes
        hook = _ntff_profile_via_ctypes("/opt/axon/libaxon_pjrt.so")
        if hook is not None:
            mod._hook = hook
    except Exception:
        pass


def _get_nc():
    key = (MM_DT,)
    if key not in _CACHE:
        _CACHE[key] = _build_nc()
    return _CACHE[key]


def kernel(x, W, U, b, gamma, beta, moving_mean, moving_var):
    from concourse.bass_utils import run_bass_kernel_spmd
    x = np.asarray(x, np.float32)
    W = np.asarray(W, np.float32)
    U = np.asarray(U, np.float32)
    b = np.asarray(b, np.float32)
    gamma = np.asarray(gamma, np.float32)
    beta = np.asarray(beta, np.float32)
    moving_mean = np.asarray(moving_mean, np.float32)
    moving_var = np.asarray(moving_var, np.float32)
    B = x.shape[0]

    in_maps = []
    for bidx in range(B):
        for half in range(N_HALVES):
            in_maps.append(_prep_core_inputs(
                x, W, U, b, gamma, beta, moving_mean, moving_var, bidx, half))

    nc = _get_nc()
    trace = os.environ.get("BASS_KERNEL_TRACE") == "1"
    if trace:
        _install_ntff_hook()
    res = run_bass_kernel_spmd(nc, in_maps, core_ids=list(range(8)),
                               trace=trace)
    kernel._last_result = res

    out = np.zeros((B, T, H2, W2, F), np.float32)
    ci = 0
    for bidx in range(B):
        for half in range(N_HALVES):
            yc = res.results[ci]["y"].reshape(T, F, R, W2)
            ci += 1
            yc = yc.transpose(0, 2, 3, 1)  # (T, R, W2, F)
            if half == 1:
                yc = yc[:, ::-1, :, :]
                out[bidx, :, 32:64] = yc
            else:
                out[bidx, :, 0:32] = yc
    return out
